# revision 1
# baseline (speedup 1.0000x reference)
"""Trainium2 Bass kernel for Swin-style windowed attention w/ relative position bias.

Problem: x[8, 1025, 768], 12 heads, head_dim 64, rel-pos bias table gathered
by a constant index matrix. Sharding: pure data-parallel — one batch element
per NeuronCore (8 cores).

Per-core dataflow (all matmuls f32r on PE; S kept transposed so softmax
normalize and P@V need no on-chip transposes; q padded 1025->1028 so all
matmul chunks are PSUM-bank aligned and have even width):
  xT_aug [769, 1028]   (x[b].T plus a ones-row that realizes the v bias add)
  qkT = wqk_aug.T @ xT_aug      -> [1536, 1028]  (q/k per head, channels-major,
                                   head pairs share a 128-partition tile)
  v   = xT_aug.T @ wv_aug       -> [1025, 768]   (tokens-major, + ones cols)
  S.T[k,q] = k_h @ q_h.T        (K=64; head pairs run as concurrent row-tiles)
  P.T = exp(S.T + biasT)        (bias streamed bf16, DVE add + one wide ACT exp)
  [O.T; rowsum] = [v_h|1].T @ P.T   (ones col gives softmax denominators)
  1/rowsum = exp(-ln(rowsum))   (two ACT ops; keeps the slow DVE reciprocal off
                                 the critical path)
  O.T *= (1/rowsum) broadcast   (K=1 ones matmul broadcast + DVE mult)
  outT = wproj.T @ O_all + proj_b   -> [768, 1025], host transposes back
"""

import sys

import numpy as np

for _p in ("/opt/trn_rl_repo",):
    if _p not in sys.path:
        sys.path.insert(0, _p)

B = 8
N = 1025
NP = 1028        # q padded to 2 full banks + one 4-wide tail chunk
C = 768
H = 12
D = 64
SCALE = D ** -0.5
NKT = 9          # k tiles of 128 (8 full + 1)
QCHUNKS = [(0, 512), (512, 512), (1024, 4)]
NCHUNKS_V = [(0, 512), (512, 256)]


def _emit(ctx, tc, xT_aug, wqk_aug, wqkb, wv_aug, bias_t, wproj, projb, outT):
    import concourse.mybir as mybir

    nc = tc.nc
    f32 = mybir.dt.float32
    f32r = mybir.dt.float32r
    bf16 = mybir.dt.bfloat16
    AF = mybir.ActivationFunctionType

    def ktsize(kt):
        return 128 if kt < 8 else 1

    lp = nc.allow_low_precision(
        reason="float32r is fp32-width storage; PE rounding only")
    lp.__enter__()
    ctx.callback(lambda: lp.__exit__(None, None, None))

    # Long-lived pools first (pool release must be LIFO / stack ordered).
    qk_pool = ctx.enter_context(tc.tile_pool(name="qk_pool", bufs=1))
    v_pool = ctx.enter_context(tc.tile_pool(name="v_pool", bufs=1))
    qk_sb = []
    for m in range(12):
        t = qk_pool.tile([128, NP], f32r, tag=f"qk{m}", name=f"qk{m}")
        qk_sb.append(t)
    v_sb = []
    for kt in range(NKT):
        t = v_pool.tile([128, H, D + 1], f32r, tag=f"v{kt}", name=f"v{kt}")
        v_sb.append(t)
        nc.vector.memset(t[:, :, D:D + 1].bitcast(f32), 1.0)
        nc.scalar.copy(t[:, :, D:D + 1], t[:, :, D:D + 1].bitcast(f32))

    # bias stream pool before phase-1 temporaries: its addresses must not
    # overlap phase-1 tiles so the DMAs can prefetch during the projections.
    bpool = ctx.enter_context(tc.tile_pool(name="bpool", bufs=7))

    # ---------------- phase 1: load weights + x, QKV projections -------------
    xpool = tc.alloc_tile_pool(name="xpool", bufs=1)
    wvpool = tc.alloc_tile_pool(name="wvpool", bufs=1)
    wqkpool = tc.alloc_tile_pool(name="wqkpool", bufs=1)

    wqk_t, wv_t, xT_t = [], [], []
    for ct in range(7):
        p = 128 if ct < 6 else 1
        xt = xpool.tile([p, NP], f32r, tag=f"xT{ct}", name=f"xT{ct}")
        nc.sync.dma_start(xt[:, :], xT_aug[ct * 128: ct * 128 + p, :])
        xT_t.append(xt)
        w2 = wvpool.tile([p, 768], f32r, tag=f"wv{ct}", name=f"wv{ct}")
        nc.sync.dma_start(w2[:, :], wv_aug[ct * 128: ct * 128 + p, :])
        wv_t.append(w2)
        if ct < 6:
            w1 = wqkpool.tile([p, 1536], f32r, tag=f"wqk{ct}",
                              name=f"wqk{ct}")
            nc.sync.dma_start(w1[:, :], wqk_aug[ct * 128:(ct + 1) * 128, :])
            wqk_t.append(w1)
    wqkb_t = wqkpool.tile([128, 12], f32, tag="wqkb", name="wqkb")
    nc.sync.dma_start(wqkb_t[:, :], wqkb[:, :])

    with tc.tile_pool(name="ps1", bufs=4, space="PSUM") as ps1, \
         tc.tile_pool(name="ps1v", bufs=2, space="PSUM") as ps1v:
        # v first (needs only xT+wv, ~5.5MB of DMA): PE starts sooner.
        # v: v[n-tile, c'] = sum_ct xT[ct, n-tile].T @ wv[ct, c']
        for kt in range(NKT):
            p = ktsize(kt)
            ps = ps1v.tile([128, 768], f32, tag="ps1v", name=f"ps1v_{kt}")
            for (c0, cn) in NCHUNKS_V:
                for ct in range(7):
                    nc.tensor.matmul(
                        ps[:p, c0:c0 + cn],
                        xT_t[ct][:, kt * 128: kt * 128 + p],
                        wv_t[ct][:, c0:c0 + cn],
                        start=(ct == 0), stop=(ct == 6),
                    )
            nc.scalar.copy(
                v_sb[kt][:p, :, 0:D],
                ps[:p, :].rearrange("p (h d) -> p h d", h=H),
            )
        # q/k: qkT[m-tile, n] = sum_ct wqk[ct, m-tile].T @ xT[ct, n]
        for m in range(12):
            for (q0, qn) in QCHUNKS:
                ps = ps1.tile([128, 512], f32, tag="ps1t", name=f"ps1_{m}_{q0}")
                for ct in range(6):
                    nc.tensor.matmul(
                        ps[:, :qn],
                        wqk_t[ct][:, m * 128:(m + 1) * 128],
                        xT_t[ct][:, q0:q0 + qn],
                        start=(ct == 0), stop=(ct == 5),
                    )
                nc.scalar.activation(qk_sb[m][:, q0:q0 + qn], ps[:, :qn],
                                     AF.Identity, bias=wqkb_t[:, m:m + 1])

    wqkpool.release()
    wvpool.release()
    xpool.release()

    # ---------------- phase 2: attention, one head pair at a time ------------
    ppool = ctx.enter_context(tc.tile_pool(name="ppool", bufs=6))
    npool = ctx.enter_context(tc.tile_pool(name="npool", bufs=3))
    cpool = ctx.enter_context(tc.tile_pool(name="cpool", bufs=1))
    ones64 = cpool.tile([1, 64], f32r)
    nc.vector.memset(ones64[:, :].bitcast(f32), 1.0)
    nc.scalar.copy(ones64[:, :], ones64[:, :].bitcast(f32))
    opool = ctx.enter_context(tc.tile_pool(name="opool", bufs=1))
    o_all = []
    for m in range(6):
        t = opool.tile([128, NP], f32r, tag=f"oall{m}", name=f"oall{m}")
        o_all.append(t)

    wp_pool = ctx.enter_context(tc.tile_pool(name="wp_pool", bufs=1))
    wproj_t = []
    projb_t = []
    for ct in range(6):
        t = wp_pool.tile([128, 768], f32r, tag=f"wproj{ct}",
                         name=f"wproj{ct}")
        nc.sync.dma_start(t[:, :], wproj[ct * 128:(ct + 1) * 128, :])
        wproj_t.append(t)
        tb = wp_pool.tile([128, 1], f32, tag=f"projb{ct}", name=f"projb{ct}")
        nc.sync.dma_start(tb[:, :], projb[ct * 128:(ct + 1) * 128, :])
        projb_t.append(tb)

    LAG = 3
    pending_tails = []

    with tc.tile_pool(name="ps_s", bufs=3, space="PSUM") as ps_s, \
         tc.tile_pool(name="ps_o", bufs=4, space="PSUM") as ps_o:

        def make_tail(pair, hh, ci, o_ps_tile):
            # two stages: stage1 produces the broadcast reciprocal (ACT/PE),
            # stage2 is only the DVE mul — emitted a couple of units later so
            # it never head-of-line-blocks the DVE add stream.
            h = 2 * pair + hh
            q0, qn = QCHUNKS[ci]
            box = {}

            def stage1():
                lns = npool.tile([1, 512], f32, tag="lns",
                                 name=f"lns_{h}_{ci}")
                nc.scalar.activation(lns[:, :qn], o_ps_tile[64:65, :qn],
                                     AF.Ln)
                rcr = npool.tile([1, 512], f32r, tag="rcr",
                                 name=f"rcr_{h}_{ci}")
                nc.scalar.activation(rcr[:, :qn], lns[:, :qn], AF.Exp,
                                     scale=-1.0)
                bc_ps = ps_s.tile([64, 512], f32, tag="s_ps",
                                  name=f"bcps_{h}_{ci}")
                nc.tensor.matmul(bc_ps[:, :qn], ones64[:, :], rcr[:, :qn],
                                 start=True, stop=True)
                bc = npool.tile([64, 512], f32, tag="bc", name=f"bc_{h}_{ci}")
                nc.scalar.copy(bc[:, :qn], bc_ps[:, :qn])
                box["bc"] = bc

            def stage2():
                nc.vector.tensor_mul(
                    o_all[pair][hh * 64:hh * 64 + 64, q0:q0 + qn],
                    o_ps_tile[0:64, :qn],
                    box.pop("bc")[:, :qn],
                )

            return (stage1, stage2)

        for pair in range(6):
            h0 = 2 * pair
            q_t = qk_sb[2 * pair]
            k_t = qk_sb[2 * pair + 1]
            o_ps = {}
            for hh in (0, 1):
                for ci in range(2):
                    o_ps[(hh, ci)] = ps_o.tile(
                        [65, 512], f32, tag="o_ps", name=f"ops_{h0 + hh}_{ci}")
            o_c2 = ps_o.tile([65, 8], f32, tag="o_c2", name=f"oc2_{h0}",
                             bufs=1)
            o_ps[(0, 2)] = o_c2[:, 0:4]
            o_ps[(1, 2)] = o_c2[:, 4:8]
            pes = {}

            def s_unit(kt):
                # S.T matmuls (head pair = concurrent PE row-tiles), bias
                # add, and one wide exp per head.
                p = ktsize(kt)
                bts = []
                for hh in (0, 1):
                    bt = bpool.tile([128, NP], bf16, tag="bt",
                                    name=f"bt_{h0 + hh}_{kt}")
                    nc.sync.dma_start(bt[:p, :], bias_t[h0 + hh, kt, 0:p, :])
                    bts.append(bt)
                pts = []
                for hh in (0, 1):
                    pt = ppool.tile([128, NP], f32, tag="pt",
                                    name=f"pt_{h0 + hh}_{kt}", bufs=4)
                    pts.append(pt)
                for ci, (q0, qn) in enumerate(QCHUNKS):
                    for hh in (0, 1):
                        pr0 = hh * 64
                        s_ps = ps_s.tile([128, 512], f32, tag="s_ps",
                                         name=f"sps_{h0 + hh}_{kt}_{ci}")
                        nc.tensor.matmul(
                            s_ps[:p, :qn],
                            k_t[pr0:pr0 + 64, kt * 128: kt * 128 + p],
                            q_t[pr0:pr0 + 64, q0:q0 + qn],
                            start=True, stop=True,
                        )
                        nc.vector.tensor_add(
                            pts[hh][:p, q0:q0 + qn],
                            s_ps[:p, :qn], bts[hh][:p, q0:q0 + qn])
                for hh in (0, 1):
                    pe = ppool.tile([128, NP], f32r, tag="pe",
                                    name=f"pe_{h0 + hh}_{kt}", bufs=8)
                    nc.scalar.activation(pe[:p, :], pts[hh][:p, :], AF.Exp)
                    pes[(kt, hh)] = pe

            def pv_unit(kt):
                p = ktsize(kt)
                for ci, (q0, qn) in enumerate(QCHUNKS):
                    for hh in (0, 1):
                        # ci==2: both heads share one PSUM bank (pre-zeroed
                        # by DVE), so never use start=True there
                        nc.tensor.matmul(
                            o_ps[(hh, ci)][:, :qn],
                            v_sb[kt][:p, h0 + hh, :],
                            pes[(kt, hh)][:p, q0:q0 + qn],
                            start=(kt == 0 and ci != 2), stop=(kt == 8),
                            skip_group_check=(ci == 2),
                        )
                for hh in (0, 1):
                    pes.pop((kt, hh))

            stage2s = []
            for kt in range(NKT):
                s_unit(kt)
                if kt == LAG - 1:
                    # shared-bank tail accumulator must be zeroed before the
                    # first PV of this pair touches it (at kt == LAG)
                    nc.vector.memset(o_c2[:, :], 0.0)
                if kt >= LAG:
                    pv_unit(kt - LAG)
                # previous pair's normalize tails: stage1 (ACT/PE) early,
                # the DVE mul two units later so bc is long ready
                for _ in range(2):
                    if pending_tails:
                        s1, s2 = pending_tails.pop(0)
                        s1()
                        stage2s.append(s2)
                if kt >= 2:
                    for _ in range(2):
                        if stage2s:
                            stage2s.pop(0)()
            for kt in range(NKT - LAG, NKT):
                pv_unit(kt)
                for _ in range(2):
                    if stage2s:
                        stage2s.pop(0)()
            for s2 in stage2s:
                s2()

            for ci in range(3):
                for hh in (0, 1):
                    pending_tails.append(
                        make_tail(pair, hh, ci, o_ps[(hh, ci)]))

        for s1, s2 in pending_tails:
            s1()
            s2()

    # ---------------- phase 3: output projection ----------------
    with tc.tile_pool(name="ops3", bufs=3, space="PSUM") as ps3, \
         tc.tile_pool(name="out_pool", bufs=3) as out_pool:
        for m in range(6):
            for (q0, qn) in QCHUNKS:
                ps = ps3.tile([128, 512], f32, tag="ps3", name=f"ps3_{m}_{q0}")
                for ct in range(6):
                    nc.tensor.matmul(
                        ps[:, :qn],
                        wproj_t[ct][:, m * 128:(m + 1) * 128],
                        o_all[ct][:, q0:q0 + qn],
                        start=(ct == 0), stop=(ct == 5),
                    )
                wn = min(qn, N - q0)
                ot = out_pool.tile([128, 512], f32, tag="ot",
                                   name=f"ot_{m}_{q0}")
                nc.scalar.activation(ot[:, :wn], ps[:, :wn], AF.Identity,
                                     bias=projb_t[m])
                nc.sync.dma_start(outT[m * 128:(m + 1) * 128, q0:q0 + wn],
                                  ot[:, :wn])


def _host_prep(x, qkv_w, q_bias, v_bias, rpb_table, proj_w, proj_b,
               rel_pos_index):
    """Layout-only transforms; all FLOPs stay on device."""
    import ml_dtypes
    f = np.float32
    x = np.asarray(x, f)
    qkv_w = np.asarray(qkv_w, f)
    q_bias = np.asarray(q_bias, f)
    v_bias = np.asarray(v_bias, f)
    rpb_table = np.asarray(rpb_table, f)
    proj_w = np.asarray(proj_w, f)
    proj_b = np.asarray(proj_b, f)
    idx = np.asarray(rel_pos_index)

    # q/k weights: column blocks [q0 q1 | k0 k1 | q2 q3 | k2 k3 | ...],
    # q pre-scaled by 1/sqrt(D); q/k biases added at PSUM evacuation.
    rows = []
    brows = []
    for p in range(6):
        rows.append(qkv_w[p * 128:(p + 1) * 128] * SCALE)
        brows.append(q_bias[p * 128:(p + 1) * 128] * SCALE)
        rows.append(qkv_w[C + p * 128: C + (p + 1) * 128])
        brows.append(np.zeros(128, f))
    wqk = np.concatenate(rows, axis=0)               # [1536, 768]
    wqk_bias = np.concatenate(brows, axis=0)         # [1536]
    wqk_aug = np.ascontiguousarray(wqk.T)            # [768, 1536]
    wqkb = np.ascontiguousarray(wqk_bias.reshape(12, 128).T)  # [128, 12]

    wv_aug = np.concatenate(
        [qkv_w[2 * C:3 * C].T, v_bias[None, :]], axis=0)      # [769, 768]

    rpb = rpb_table[idx]                              # [N, N, H] (q, k, h)
    biasT = np.ascontiguousarray(rpb.transpose(2, 1, 0))  # [H, k, q]
    bias_pad = np.zeros((H, NKT * 128, NP), f)
    bias_pad[:, :N, :N] = biasT
    bias_t = bias_pad.reshape(H, NKT, 128, NP).astype(ml_dtypes.bfloat16)

    wproj = np.ascontiguousarray(proj_w.T)            # [768, 768]
    projb = np.ascontiguousarray(proj_b.reshape(C, 1))

    xT_aug = np.zeros((B, 769, NP), f)
    for b in range(B):
        xT_aug[b, :C, :N] = x[b].T
    xT_aug[:, C, :] = 1.0                             # bias row (ones)
    return xT_aug, wqk_aug, wqkb, wv_aug, bias_t, wproj, projb


_BUILT = {}


def _build():
    if "nc" in _BUILT:
        return _BUILT["nc"]
    from contextlib import ExitStack

    import concourse.mybir as mybir
    import concourse.tile as tile
    from concourse import bacc

    nc = bacc.Bacc("TRN2", target_bir_lowering=False, debug=False,
                   num_devices=B)
    f32 = mybir.dt.float32
    f32r = mybir.dt.float32r
    xT_aug = nc.dram_tensor("xT_aug", (769, NP), f32r,
                            kind="ExternalInput").ap()
    wqk_aug = nc.dram_tensor("wqk_aug", (768, 1536), f32r,
                             kind="ExternalInput").ap()
    wqkb = nc.dram_tensor("wqkb", (128, 12), f32, kind="ExternalInput").ap()
    wv_aug = nc.dram_tensor("wv_aug", (769, 768), f32r,
                            kind="ExternalInput").ap()
    bias_t = nc.dram_tensor("bias_t", (H, NKT, 128, NP), mybir.dt.bfloat16,
                            kind="ExternalInput").ap()
    wproj = nc.dram_tensor("wproj", (768, 768), f32r,
                           kind="ExternalInput").ap()
    projb = nc.dram_tensor("projb", (768, 1), f32, kind="ExternalInput").ap()
    outT = nc.dram_tensor("outT", (768, N), f32, kind="ExternalOutput").ap()

    with tile.TileContext(nc) as tc:
        with ExitStack() as ctx:
            _emit(ctx, tc, xT_aug, wqk_aug, wqkb, wv_aug, bias_t, wproj,
                  projb, outT)
    nc.compile()
    _BUILT["nc"] = nc
    return nc


def kernel(x, qkv_w, q_bias, v_bias, rpb_table, proj_w, proj_b,
           rel_pos_index):
    from concourse.bass_utils import run_bass_kernel_spmd

    xT_aug, wqk_aug, wqkb, wv_aug, bias_t, wproj, projb = _host_prep(
        x, qkv_w, q_bias, v_bias, rpb_table, proj_w, proj_b, rel_pos_index)

    nc = _build()
    shared = {
        "wqk_aug": wqk_aug, "wqkb": wqkb, "wv_aug": wv_aug, "bias_t": bias_t,
        "wproj": wproj, "projb": projb,
    }
    in_maps = [dict(shared, xT_aug=np.ascontiguousarray(xT_aug[b]))
               for b in range(B)]
    res = run_bass_kernel_spmd(nc, in_maps, core_ids=list(range(B)))
    out = np.stack([res.results[b]["outT"].T for b in range(B)], axis=0)
    return out.astype(np.float32)



# revision 10
# speedup vs baseline: 1.2426x; 1.2426x over previous
"""Trainium2 Bass kernel for Swin-style windowed attention w/ relative position bias.

Problem: x[8, 1025, 768], 12 heads, head_dim 64, rel-pos bias table gathered
by a constant index matrix. Sharding: pure data-parallel - one batch element
per NeuronCore (8 cores).

v2 design (vs f32r baseline at 492us):
  - all matmul inputs fp16: PE runs 1 cyc/row instead of ~3 (fp32 HIGH mode),
    10-bit mantissa keeps rel-err ~1e-3
  - one manual InstLoadActFuncSet(natural_log_exp_and_others) at kernel
    start: kills the 49 Exp<->Ln ACT table loads (62us) the auto-pass
    inserted
  - rel-pos bias is DVE-copied (2x mode, fp16->f32) into PSUM and the S
    matmul accumulates on top (start=False): removes the 324 slow 1x
    PSUM-source tensor_add ops; exp reads PSUM directly
  - PSUM evacuations (qk, v, proj out) moved from ACT to DVE tensor_scalar
    ops (ACT is exp-bound)
  - q processed as 2x512 chunks + a single tail column (q=1024) through a
    shared tiny PSUM region, instead of 4-wide third chunks everywhere

Per-core dataflow (S kept transposed [k, q] so softmax normalize and P@V
need no on-chip transposes):
  xT_aug [769, 1028]   (x[b].T plus a ones-row that realizes the v bias add)
  qkT = wqk_aug.T @ xT_aug      -> [1536, 1028]  (q/k per head, channels-major,
                                   head pairs share a 128-partition tile)
  v   = xT_aug.T @ wv_aug       -> [1025, 768]   (tokens-major, + ones cols)
  PSUM <- bias.T (DVE copy), PSUM += k_h @ q_h.T (accumulating matmul)
  P.T = exp(PSUM)               (one ACT op per 512-chunk, PSUM -> SBUF fp16)
  [O.T; rowsum] = [v_h|1].T @ P.T   (ones col gives softmax denominators)
  1/rowsum = exp(-ln(rowsum))   (ACT, both fns resident in one table set)
  O.T *= (1/rowsum) broadcast   (K=1 ones matmul broadcast + DVE mult)
  outT = wproj.T @ O_all + proj_b   -> [768, 1025], host transposes back
"""

import sys

import numpy as np

for _p in ("/opt/trn_rl_repo",):
    if _p not in sys.path:
        sys.path.insert(0, _p)

B = 8
N = 1025
C = 768
H = 12
D = 64
SCALE = D ** -0.5
NKT = 9          # k tiles of 128 (8 full + 1)
NPQ = 1028       # qk/bias tile width (1025 used, padded for alignment)
QC = [(0, 512), (512, 512)]   # main q chunks
QT = 1024        # tail q column
LAG = 3

# bisect/config knobs (module-level so tests can flip before _build())
PIN_ACT_TABLE = True    # manual covering-set InstLoadActFuncSet at start
USE_FP16 = True         # False -> bfloat16 for all 16-bit tiles
PRELOAD_BIAS = True     # False -> baseline-style DVE tensor_add of bias
TAILS = True            # False -> skip the q=1024 tail pipeline entirely
DVE_EVAC = True         # False -> ACT (scalar engine) evacuations


def _emit(ctx, tc, xT_aug, wqk_aug, wqkb, wv_aug, bias_t, bias_tl, wproj,
          projb, outT, act_set_id):
    import concourse.mybir as mybir

    nc = tc.nc
    f32 = mybir.dt.float32
    f16 = mybir.dt.float16 if USE_FP16 else mybir.dt.bfloat16
    AF = mybir.ActivationFunctionType

    def ktsize(kt):
        return 128 if kt < 8 else 1

    lp = nc.allow_low_precision(
        reason="fp16 matmul inputs with fp32 PSUM accumulation")
    lp.__enter__()
    ctx.callback(lambda: lp.__exit__(None, None, None))

    if PIN_ACT_TABLE:
        # Pin the one ACT table set containing Exp+Ln+Copy+Identity so the
        # auto-insertion pass sees every path pre-loaded and adds no loads.
        nc.scalar.add_instruction(mybir.InstLoadActFuncSet(
            name=nc.get_next_instruction_name(), act_func_set_id=act_set_id,
            ins=[], outs=[]))

    # Long-lived pools first (pool release must be LIFO / stack ordered).
    qk_pool = ctx.enter_context(tc.tile_pool(name="qk_pool", bufs=1))
    v_pool = ctx.enter_context(tc.tile_pool(name="v_pool", bufs=1))
    qk_sb = []
    for m in range(12):
        t = qk_pool.tile([128, NPQ], f16, tag=f"qk{m}", name=f"qk{m}")
        qk_sb.append(t)
    v_sb = []
    for kt in range(NKT):
        t = v_pool.tile([128, H, 66], f16, tag=f"v{kt}", name=f"v{kt}")
        v_sb.append(t)
        nc.vector.memset(t[:, :, 64:65], 1.0)

    btl_pool = ctx.enter_context(tc.tile_pool(name="btl_pool", bufs=1))
    btl = None
    if TAILS:
        btl = btl_pool.tile([128, 6, NKT, 4], f16, tag="btl", name="btl")
        nc.sync.dma_start(btl[:, :, :, :], bias_tl[:, :, :, :])

    # bias stream pool before phase-1 temporaries: its addresses must not
    # overlap phase-1 tiles so the DMAs can prefetch during the projections.
    bpool = ctx.enter_context(tc.tile_pool(name="bpool", bufs=6))

    # ---------------- phase 1: load weights + x, QKV projections -------------
    xpool = tc.alloc_tile_pool(name="xpool", bufs=1)
    wvpool = tc.alloc_tile_pool(name="wvpool", bufs=1)
    wqkpool = tc.alloc_tile_pool(name="wqkpool", bufs=1)

    wqk_t, wv_t, xT_t = [], [], []
    for ct in range(7):
        p = 128 if ct < 6 else 1
        xt = xpool.tile([p, NPQ], f16, tag=f"xT{ct}", name=f"xT{ct}")
        nc.sync.dma_start(xt[:, :], xT_aug[ct * 128: ct * 128 + p, :])
        xT_t.append(xt)
        w2 = wvpool.tile([p, 768], f16, tag=f"wv{ct}", name=f"wv{ct}")
        nc.sync.dma_start(w2[:, :], wv_aug[ct * 128: ct * 128 + p, :])
        wv_t.append(w2)
        if ct < 6:
            w1 = wqkpool.tile([p, 1536], f16, tag=f"wqk{ct}",
                              name=f"wqk{ct}")
            nc.sync.dma_start(w1[:, :], wqk_aug[ct * 128:(ct + 1) * 128, :])
            wqk_t.append(w1)
    wqkb_t = wqkpool.tile([128, 12], f32, tag="wqkb", name="wqkb")
    nc.sync.dma_start(wqkb_t[:, :], wqkb[:, :])

    with tc.tile_pool(name="ps1", bufs=1, space="PSUM") as ps1, \
         tc.tile_pool(name="ps1v", bufs=1, space="PSUM") as ps1v:
        # v first (needs only xT+wv): PE starts sooner.
        # v: v[n-tile, c'] = sum_ct xT[ct, n-tile].T @ wv[ct, c']
        for kt in range(NKT):
            p = ktsize(kt)
            ps = ps1v.tile([128, 768], f32, tag="ps1v", name=f"ps1v_{kt}",
                           bufs=2)
            for (c0, cn) in [(0, 512), (512, 256)]:
                for ct in range(7):
                    nc.tensor.matmul(
                        ps[:p, c0:c0 + cn],
                        xT_t[ct][:, kt * 128: kt * 128 + p],
                        wv_t[ct][:, c0:c0 + cn],
                        start=(ct == 0), stop=(ct == 6),
                    )
            if DVE_EVAC:
                nc.vector.tensor_copy(
                    v_sb[kt][:p, :, 0:64],
                    ps[:p, :].rearrange("p (h d) -> p h d", h=H),
                )
            else:
                nc.scalar.copy(
                    v_sb[kt][:p, :, 0:64],
                    ps[:p, :].rearrange("p (h d) -> p h d", h=H),
                )
        # q/k: qkT[m-tile, n] = sum_ct wqk[ct, m-tile].T @ xT[ct, n]
        for m in range(12):
            for (q0, qn) in QC:
                ps = ps1.tile([128, 512], f32, tag="ps1t",
                              name=f"ps1_{m}_{q0}", bufs=3)
                for ct in range(6):
                    nc.tensor.matmul(
                        ps[:, :qn],
                        wqk_t[ct][:, m * 128:(m + 1) * 128],
                        xT_t[ct][:, q0:q0 + qn],
                        start=(ct == 0), stop=(ct == 5),
                    )
                if DVE_EVAC:
                    nc.vector.tensor_scalar_add(
                        qk_sb[m][:, q0:q0 + qn], ps[:, :qn],
                        wqkb_t[:, m:m + 1])
                else:
                    nc.scalar.activation(qk_sb[m][:, q0:q0 + qn], ps[:, :qn],
                                         AF.Identity,
                                         bias=wqkb_t[:, m:m + 1])
            if TAILS:
                pst = ps1.tile([128, 2], f32, tag="ps1tt", name=f"ps1t_{m}",
                               bufs=1)
                for ct in range(6):
                    nc.tensor.matmul(
                        pst[:, 0:2],
                        wqk_t[ct][:, m * 128:(m + 1) * 128],
                        xT_t[ct][:, QT:QT + 2],
                        start=(ct == 0), stop=(ct == 5),
                    )
                nc.vector.tensor_scalar_add(
                    qk_sb[m][:, QT:QT + 2], pst[:, 0:2], wqkb_t[:, m:m + 1])

    wqkpool.release()
    wvpool.release()
    xpool.release()

    # ---------------- phase 2: attention, one head pair at a time ------------
    ppool = ctx.enter_context(tc.tile_pool(name="ppool", bufs=8))
    npool = ctx.enter_context(tc.tile_pool(name="npool", bufs=3))
    bcpool = ctx.enter_context(tc.tile_pool(name="bcpool", bufs=3))
    cpool = ctx.enter_context(tc.tile_pool(name="cpool", bufs=1))
    ones64 = cpool.tile([1, 64], f16, tag="ones", name="ones64")
    nc.vector.memset(ones64[:, :], 1.0)
    opool = ctx.enter_context(tc.tile_pool(name="opool", bufs=1))
    o_all = []
    for m in range(6):
        t = opool.tile([128, NPQ], f16, tag=f"oall{m}", name=f"oall{m}")
        o_all.append(t)

    wp_pool = ctx.enter_context(tc.tile_pool(name="wp_pool", bufs=1))
    wproj_t = []
    projb_t = []
    for ct in range(6):
        t = wp_pool.tile([128, 768], f16, tag=f"wproj{ct}",
                         name=f"wproj{ct}")
        nc.sync.dma_start(t[:, :], wproj[ct * 128:(ct + 1) * 128, :])
        wproj_t.append(t)
        tb = wp_pool.tile([128, 1], f32, tag=f"projb{ct}", name=f"projb{ct}")
        nc.sync.dma_start(tb[:, :], projb[ct * 128:(ct + 1) * 128, :])
        projb_t.append(tb)

    pending = []

    with tc.tile_pool(name="ps_s", bufs=1, space="PSUM") as ps_s, \
         tc.tile_pool(name="ps_o", bufs=1, space="PSUM") as ps_o:

        def make_norm(pair, hh, ci, o_t):
            # stage1 produces the broadcast reciprocal (ACT/PE/DVE); stage2
            # is the final DVE mul, emitted a couple of units later so bc
            # never head-of-line-blocks the DVE stream.
            q0, qn = QC[ci]
            box = {}

            def s1():
                lns = npool.tile([1, 512], f32, tag="lns",
                                 name=f"lns_{pair}_{hh}_{ci}")
                nc.scalar.activation(lns[:, :qn], o_t[64:65, :qn], AF.Ln)
                rcr = npool.tile([1, 512], f16, tag="rcr",
                                 name=f"rcr_{pair}_{hh}_{ci}")
                nc.scalar.activation(rcr[:, :qn], lns[:, :qn], AF.Exp,
                                     scale=-1.0)
                bc_ps = ps_s.tile([64, 512], f32, tag="s", bufs=3,
                                  name=f"bcps_{pair}_{hh}_{ci}")
                nc.tensor.matmul(bc_ps[:, :qn], ones64[:, :], rcr[:, :qn],
                                 start=True, stop=True)
                bc = bcpool.tile([64, 512], f16, tag="bc",
                                 name=f"bc_{pair}_{hh}_{ci}")
                nc.vector.tensor_copy(bc[:, :qn], bc_ps[:, :qn])
                box["bc"] = bc

            def s2():
                nc.vector.tensor_mul(
                    o_all[pair][hh * 64:hh * 64 + 64, q0:q0 + qn],
                    o_t[0:64, :qn],
                    box.pop("bc")[:, :qn],
                )

            return (s1, s2)

        def make_norm_tail(pair, T):
            box = {}

            def s1():
                lnt = npool.tile([1, 4], f32, tag="lnt",
                                 name=f"lnt_{pair}")
                nc.scalar.activation(lnt[:, 0:4], T[64:65, 0:4], AF.Ln)
                rct = npool.tile([1, 4], f16, tag="rct",
                                 name=f"rct_{pair}")
                nc.scalar.activation(rct[:, 0:4], lnt[:, 0:4], AF.Exp,
                                     scale=-1.0)
                bc_ps = ps_s.tile([64, 4], f32, tag="s", bufs=3,
                                  name=f"bctp_{pair}")
                for hh in (0, 1):
                    nc.tensor.matmul(bc_ps[:, 2 * hh:2 * hh + 2],
                                     ones64[:, :], rct[:, 2 * hh:2 * hh + 2],
                                     start=True, stop=True)
                bc = bcpool.tile([64, 4], f16, tag="bct",
                                 name=f"bct_{pair}")
                nc.vector.tensor_copy(bc[:, 0:4], bc_ps[:, 0:4])
                box["bc"] = bc

            def s2():
                bc = box.pop("bc")
                for hh in (0, 1):
                    nc.vector.tensor_mul(
                        o_all[pair][hh * 64:hh * 64 + 64, QT:QT + 2],
                        T[0:64, 2 * hh:2 * hh + 2],
                        bc[:, 2 * hh:2 * hh + 2],
                    )

            return (s1, s2)

        for pair in range(6):
            h0 = 2 * pair
            q_t = qk_sb[2 * pair]
            k_t = qk_sb[2 * pair + 1]
            o_ps = {}
            for hh in (0, 1):
                for ci in range(2):
                    o_ps[(hh, ci)] = ps_o.tile(
                        [65, 512], f32, tag="o_ps", bufs=4,
                        name=f"ops_{h0 + hh}_{ci}")
            # PV-tail accumulator bank (cols 2hh:2hh+2 per head, rowsum in
            # row 64). Only PE touches it between the memset and the
            # pair-end ln read: PSUM forbids TensorE-write + DVE/ACT-read
            # in the same bank, even at different addresses.
            T = ps_o.tile([128, 4], f32, tag="tails", bufs=1,
                          name=f"T_{pair}")
            pes = {}
            petails = {}

            def s_unit(kt):
                # bias preload into PSUM (DVE 2x copy), S matmul accumulates
                # on top, one wide exp per 512-chunk straight from PSUM.
                p = ktsize(kt)
                bts = []
                for hh in (0, 1):
                    bt = bpool.tile([128, NPQ], f16, tag="bt",
                                    name=f"bt_{h0 + hh}_{kt}")
                    nc.sync.dma_start(bt[:p, :], bias_t[h0 + hh, kt, 0:p, :])
                    bts.append(bt)
                pe2 = {}
                for hh in (0, 1):
                    pe2[hh] = ppool.tile([128, 1024], f16, tag="pe",
                                         name=f"pe_{h0 + hh}_{kt}", bufs=8)
                for hh in (0, 1):
                    pr0 = hh * 64
                    for ci, (q0, qn) in enumerate(QC):
                        sp = ps_s.tile([128, 512], f32, tag="s", bufs=3,
                                       name=f"sps_{h0 + hh}_{kt}_{ci}")
                        if PRELOAD_BIAS:
                            nc.vector.tensor_copy(sp[:p, :qn],
                                                  bts[hh][:p, q0:q0 + qn])
                            nc.tensor.matmul(
                                sp[:p, :qn],
                                k_t[pr0:pr0 + 64, kt * 128: kt * 128 + p],
                                q_t[pr0:pr0 + 64, q0:q0 + qn],
                                start=False, stop=True, skip_group_check=True,
                            )
                            nc.scalar.activation(pe2[hh][:p, q0:q0 + qn],
                                                 sp[:p, :qn], AF.Exp)
                        else:
                            nc.tensor.matmul(
                                sp[:p, :qn],
                                k_t[pr0:pr0 + 64, kt * 128: kt * 128 + p],
                                q_t[pr0:pr0 + 64, q0:q0 + qn],
                                start=True, stop=True,
                            )
                            pts = ppool.tile([128, 512], f32, tag="pts",
                                             name=f"pts_{h0+hh}_{kt}_{ci}",
                                             bufs=4)
                            nc.vector.tensor_add(pts[:p, :qn], sp[:p, :qn],
                                                 bts[hh][:p, q0:q0 + qn])
                            nc.scalar.activation(pe2[hh][:p, q0:q0 + qn],
                                                 pts[:p, :qn], AF.Exp)
                    pes[(kt, hh)] = pe2[hh]
                if not TAILS:
                    return
                # tail cols q=1024:1026 (real + pad) in an exclusive tag-"s"
                # slot; bias preloaded, 2-wide matmul per head accumulates
                stile = ps_s.tile([128, 4], f32, tag="s", bufs=3,
                                  name=f"stl_{pair}_{kt}")
                nc.vector.tensor_copy(stile[:p, 0:4], btl[:p, pair, kt, :])
                for hh in (0, 1):
                    pr0 = hh * 64
                    nc.tensor.matmul(
                        stile[:p, 2 * hh:2 * hh + 2],
                        k_t[pr0:pr0 + 64, kt * 128: kt * 128 + p],
                        q_t[pr0:pr0 + 64, QT:QT + 2],
                        start=False, stop=True, skip_group_check=True,
                    )
                pt = ppool.tile([128, 4], f16, tag="petail",
                                name=f"pt_{pair}_{kt}", bufs=4)
                nc.scalar.activation(pt[:p, :], stile[:p, 0:4], AF.Exp)
                petails[kt] = pt

            def pv_unit(kt):
                p = ktsize(kt)
                for hh in (0, 1):
                    h = h0 + hh
                    pe = pes.pop((kt, hh))
                    for ci, (q0, qn) in enumerate(QC):
                        nc.tensor.matmul(
                            o_ps[(hh, ci)][:, :qn],
                            v_sb[kt][:p, h, 0:65],
                            pe[:p, q0:q0 + qn],
                            start=(kt == 0), stop=(kt == 8),
                        )
                if not TAILS:
                    return
                pt = petails.pop(kt)
                for hh in (0, 1):
                    h = h0 + hh
                    nc.tensor.matmul(
                        T[0:65, 2 * hh:2 * hh + 2],
                        v_sb[kt][:p, h, 0:65],
                        pt[:p, 2 * hh:2 * hh + 2],
                        start=False, stop=(kt == 8), skip_group_check=True,
                    )

            stage2s = []
            for kt in range(NKT):
                s_unit(kt)
                if kt == LAG - 1 and TAILS:
                    # PV-tail accumulator must be zeroed before the first PV
                    # of this pair touches it (at kt == LAG); by now the
                    # previous pair's readers of T have been drained
                    nc.vector.memset(T[:, 0:4], 0.0)
                if kt >= LAG:
                    pv_unit(kt - LAG)
                # previous pair's normalize: stage1 early, the DVE mul a
                # couple of units later so bc is long ready
                for _ in range(3):
                    if pending:
                        s1, s2 = pending.pop(0)
                        s1()
                        stage2s.append(s2)
                if kt >= 2:
                    for _ in range(3):
                        if stage2s:
                            stage2s.pop(0)()
            for kt in range(NKT - LAG, NKT):
                pv_unit(kt)
                for _ in range(3):
                    if stage2s:
                        stage2s.pop(0)()
            while stage2s:
                stage2s.pop(0)()

            for hh in (0, 1):
                for ci in range(2):
                    pending.append(make_norm(pair, hh, ci, o_ps[(hh, ci)]))
            if TAILS:
                pending.append(make_norm_tail(pair, T))

        for s1, s2 in pending:
            s1()
            s2()

    # ---------------- phase 3: output projection ----------------
    with tc.tile_pool(name="ps3", bufs=1, space="PSUM") as ps3, \
         tc.tile_pool(name="out_pool", bufs=3) as out_pool:
        for m in range(6):
            for (q0, qn) in QC:
                ps = ps3.tile([128, 512], f32, tag="ps3", bufs=3,
                              name=f"ps3_{m}_{q0}")
                for ct in range(6):
                    nc.tensor.matmul(
                        ps[:, :qn],
                        wproj_t[ct][:, m * 128:(m + 1) * 128],
                        o_all[ct][:, q0:q0 + qn],
                        start=(ct == 0), stop=(ct == 5),
                    )
                ot = out_pool.tile([128, 512], f32, tag="ot",
                                   name=f"ot_{m}_{q0}")
                if DVE_EVAC:
                    nc.vector.tensor_scalar_add(ot[:, :qn], ps[:, :qn],
                                                projb_t[m])
                else:
                    nc.scalar.activation(ot[:, :qn], ps[:, :qn], AF.Identity,
                                         bias=projb_t[m])
                nc.sync.dma_start(outT[m * 128:(m + 1) * 128, q0:q0 + qn],
                                  ot[:, :qn])
            if TAILS:
                pst = ps3.tile([128, 2], f32, tag="ps3t", bufs=2,
                               name=f"ps3t_{m}")
                for ct in range(6):
                    nc.tensor.matmul(
                        pst[:, 0:2],
                        wproj_t[ct][:, m * 128:(m + 1) * 128],
                        o_all[ct][:, QT:QT + 2],
                        start=(ct == 0), stop=(ct == 5),
                    )
                ott = out_pool.tile([128, 2], f32, tag="ott",
                                    name=f"ott_{m}")
                nc.vector.tensor_scalar_add(ott[:, 0:2], pst[:, 0:2],
                                            projb_t[m])
                nc.sync.dma_start(outT[m * 128:(m + 1) * 128, QT:QT + 1],
                                  ott[:, 0:1])


def _host_prep(x, qkv_w, q_bias, v_bias, rpb_table, proj_w, proj_b,
               rel_pos_index):
    """Layout-only transforms; all FLOPs stay on device."""
    f = np.float32
    if USE_FP16:
        h = np.float16
    else:
        import ml_dtypes
        h = ml_dtypes.bfloat16
    x = np.asarray(x, f)
    qkv_w = np.asarray(qkv_w, f)
    q_bias = np.asarray(q_bias, f)
    v_bias = np.asarray(v_bias, f)
    rpb_table = np.asarray(rpb_table, f)
    proj_w = np.asarray(proj_w, f)
    proj_b = np.asarray(proj_b, f)
    idx = np.asarray(rel_pos_index)

    # q/k weights: column blocks [q0 q1 | k0 k1 | q2 q3 | k2 k3 | ...],
    # q pre-scaled by 1/sqrt(D); q/k biases added at PSUM evacuation.
    rows = []
    brows = []
    for p in range(6):
        rows.append(qkv_w[p * 128:(p + 1) * 128] * SCALE)
        brows.append(q_bias[p * 128:(p + 1) * 128] * SCALE)
        rows.append(qkv_w[C + p * 128: C + (p + 1) * 128])
        brows.append(np.zeros(128, f))
    wqk = np.concatenate(rows, axis=0)               # [1536, 768]
    wqk_bias = np.concatenate(brows, axis=0)         # [1536]
    wqk_aug = np.ascontiguousarray(wqk.T).astype(h)  # [768, 1536]
    wqkb = np.ascontiguousarray(wqk_bias.reshape(12, 128).T)  # [128, 12] f32

    wv_aug = np.concatenate(
        [qkv_w[2 * C:3 * C].T, v_bias[None, :]], axis=0).astype(h)  # [769,768]

    rpb = rpb_table[idx]                              # [N, N, H] (q, k, h)
    biasT = np.ascontiguousarray(rpb.transpose(2, 1, 0))  # [H, k, q]
    bias_pad = np.zeros((H, NKT * 128, NPQ), f)
    bias_pad[:, :N, :N] = biasT
    bias_t = bias_pad.reshape(H, NKT, 128, NPQ).astype(h)
    # packed q-tail bias columns: [p, pair, kt, (b_h0, 0, b_h1, 0)]
    tl = (bias_pad[:, :, QT].reshape(H, NKT, 128).transpose(2, 0, 1)
          .reshape(128, 6, 2, NKT).transpose(0, 1, 3, 2))  # [128,6,9,2]
    bias_tl = np.zeros((128, 6, NKT, 4), f)
    bias_tl[:, :, :, 0] = tl[:, :, :, 0]
    bias_tl[:, :, :, 2] = tl[:, :, :, 1]
    bias_tl = bias_tl.astype(h)

    wproj = np.ascontiguousarray(proj_w.T).astype(h)  # [768, 768]
    projb = np.ascontiguousarray(proj_b.reshape(C, 1))

    xT_aug = np.zeros((B, 769, NPQ), f)
    for b in range(B):
        xT_aug[b, :C, :N] = x[b].T
    xT_aug[:, C, :] = 1.0                             # bias row (ones)
    xT_aug = xT_aug.astype(h)
    return xT_aug, wqk_aug, wqkb, wv_aug, bias_t, bias_tl, wproj, projb


_BUILT = {}


def _build():
    if "nc" in _BUILT:
        return _BUILT["nc"]
    from contextlib import ExitStack

    import concourse.mybir as mybir
    import concourse.tile as tile
    from concourse import bacc

    nc = bacc.Bacc("TRN2", target_bir_lowering=False, debug=False,
                   num_devices=B)
    f32 = mybir.dt.float32
    f16 = mybir.dt.float16 if USE_FP16 else mybir.dt.bfloat16
    xT_aug = nc.dram_tensor("xT_aug", (769, NPQ), f16,
                            kind="ExternalInput").ap()
    wqk_aug = nc.dram_tensor("wqk_aug", (768, 1536), f16,
                             kind="ExternalInput").ap()
    wqkb = nc.dram_tensor("wqkb", (128, 12), f32, kind="ExternalInput").ap()
    wv_aug = nc.dram_tensor("wv_aug", (769, 768), f16,
                            kind="ExternalInput").ap()
    bias_t = nc.dram_tensor("bias_t", (H, NKT, 128, NPQ), f16,
                            kind="ExternalInput").ap()
    bias_tl = nc.dram_tensor("bias_tl", (128, 6, NKT, 4), f16,
                             kind="ExternalInput").ap()
    wproj = nc.dram_tensor("wproj", (768, 768), f16,
                           kind="ExternalInput").ap()
    projb = nc.dram_tensor("projb", (768, 1), f32, kind="ExternalInput").ap()
    outT = nc.dram_tensor("outT", (768, N), f32, kind="ExternalOutput").ap()

    try:
        from concourse.hw_specs import get_activation_tables
        act_set_id = list(get_activation_tables(nc.m.arch)).index(
            "natural_log_exp_and_others")
    except Exception:
        act_set_id = 6

    with tile.TileContext(nc) as tc:
        with ExitStack() as ctx:
            _emit(ctx, tc, xT_aug, wqk_aug, wqkb, wv_aug, bias_t, bias_tl,
                  wproj, projb, outT, act_set_id)
    nc.compile()
    _BUILT["nc"] = nc
    return nc


def kernel(x, qkv_w, q_bias, v_bias, rpb_table, proj_w, proj_b,
           rel_pos_index):
    from concourse.bass_utils import run_bass_kernel_spmd

    (xT_aug, wqk_aug, wqkb, wv_aug, bias_t, bias_tl, wproj,
     projb) = _host_prep(
        x, qkv_w, q_bias, v_bias, rpb_table, proj_w, proj_b, rel_pos_index)

    nc = _build()
    shared = {
        "wqk_aug": wqk_aug, "wqkb": wqkb, "wv_aug": wv_aug, "bias_t": bias_t,
        "bias_tl": bias_tl, "wproj": wproj, "projb": projb,
    }
    in_maps = [dict(shared, xT_aug=np.ascontiguousarray(xT_aug[b]))
               for b in range(B)]
    res = run_bass_kernel_spmd(nc, in_maps, core_ids=list(range(B)))
    out = np.stack([res.results[b]["outT"].T for b in range(B)], axis=0)
    return out.astype(np.float32)
